# revision 1
# baseline (speedup 1.0000x reference)
"""Multi-head causal self-attention (B=1, S=4096, D=1024, H=16) on 8 TRN2
NeuronCores, tensor-parallel over heads (2 heads per core).

Layout strategy (everything stays in "transposed" form on device so no fp32
transposes of activations are ever needed):
  - host feeds X^T [D, S]; device computes qkv^T = (X @ Wqkv_local)^T via
    matmul(lhsT=Wqkv_tile, rhs=X^T tile).
  - scores^T [t, s] per head via matmul(lhsT=K^T tile, rhs=Q^T chunk); the two
    heads of a core occupy partitions 0-63 / 64-127 and run row-packed on the
    PE array.
  - softmax without max-subtraction (scores/8 have sigma ~0.4; exp is safe in
    fp32), exp runs on ACT straight out of PSUM with the 1/8 scale folded in.
  - P^T @ V via matmul(lhsT=V_tile[t,d] with an appended ones column, rhs=P^T)
    -> numerator rows 0-63 and the softmax denominator on row 64 of PSUM.
  - denominator: reciprocal on row 64, broadcast back to partitions 0-63 with
    a K=1 matmul against a ones row, multiply.
  - y^T partial = Wo_local^T @ out^T accumulated over both heads; each core
    writes its full [D, S] fp32 partial; host sums the 8 partials, adds bo,
    transposes.
"""

import sys

sys.path.insert(0, "/opt/trn_rl_repo")

import functools
import numpy as np
import ml_dtypes

D = 1024
H = 16
HD = 64
NCORES = 8
HPC = H // NCORES  # heads per core = 2
P = 128
CH = 512  # s-chunk width
GROUP = 2  # score slots per exp group ([128, 1024] = 2 PSUM banks)

BF16 = ml_dtypes.bfloat16


def build_nc(S):
    import concourse.bacc as bacc
    import concourse.mybir as mybir
    from concourse import tile

    f32 = mybir.dt.float32
    f32r = mybir.dt.float32r
    bf16 = mybir.dt.bfloat16
    ADD = mybir.AluOpType.add
    EXP = mybir.ActivationFunctionType.Exp

    NCHUNK = S // CH
    NT = S // P  # number of 128-row t-tiles
    ND = D // P  # 8 d-tiles

    nc = bacc.Bacc("TRN2", target_bir_lowering=False, debug=False)

    xt_d = nc.dram_tensor("xt", [D, S], f32r, kind="ExternalInput")
    wqkv_d = nc.dram_tensor("wqkv", [D, 3 * HPC * HD], f32r, kind="ExternalInput")
    bqkv_d = nc.dram_tensor("bqkv", [3 * HPC * HD], f32, kind="ExternalInput")
    wo_d = nc.dram_tensor("wo", [HPC * HD, D], f32r, kind="ExternalInput")
    masks_d = nc.dram_tensor("masks", [P, 4, CH], bf16, kind="ExternalInput")
    ones_d = nc.dram_tensor("ones", [1, HD], f32r, kind="ExternalInput")
    yt_d = nc.dram_tensor("yt", [D, S], f32, kind="ExternalOutput")

    with tile.TileContext(nc) as tc:
        with (
            tc.tile_pool(name="consts", bufs=1) as consts,
            tc.tile_pool(name="xtp", bufs=18) as xtp,
            tc.tile_pool(name="ptp", bufs=8) as ptp,
            tc.tile_pool(name="otp", bufs=6) as otp,
            tc.tile_pool(name="rcp", bufs=4) as rcp,
            tc.tile_pool(name="vtp", bufs=2) as vtp,
            tc.tile_pool(name="ytp", bufs=8) as ytp,
            tc.tile_pool(name="scp", bufs=2, space="PSUM") as scp,
            tc.tile_pool(name="avp", bufs=2, space="PSUM") as avp,
            tc.tile_pool(name="qyp", bufs=2, space="PSUM") as qyp,
        ):
            # ---- persistent SBUF (wq first: first QKV matmuls need it) ----
            wq_sb = consts.tile([P, ND, 3 * HPC * HD], f32r)
            bq_sb = consts.tile([P, 3], f32)
            wo_sb = consts.tile([HPC * HD, D], f32r)
            masks_sb = consts.tile([P, 4, CH], bf16)
            ones_sb = consts.tile([P, HD], f32r)
            nc.sync.dma_start(bq_sb[:], bqkv_d[:].rearrange("(i p) -> p i", p=P))

            qt_sb = consts.tile([P, S], f32r)  # Q^T: h0 parts 0-63, h1 64-127
            kt_sb = consts.tile([P, S], f32r)
            vt_sb = consts.tile([P, S], f32)  # V^T
            # V-hat per head: [t-part, NT tiles, 72] (cols 0-63 = V, 64 = ones)
            vhat = [
                consts.tile([P, NT, 72], f32r, tag=f"vhat{h}", name=f"vhat{h}")
                for h in range(HPC)
            ]
            for h in range(HPC):
                nc.sync.dma_start(
                    vhat[h][:, :, 64:65],
                    ones_d[0:1, 0:NT].broadcast_to([P, NT]),
                )

            vt_r = vt_sb[:].rearrange("p (jt b f) -> p jt b f", b=4, f=32)

            def emit_qkv(j):
                """QKV^T projection for s-chunk j."""
                xts = []
                for d in range(ND):
                    if j == 0:  # interleave weight loads with the first x tiles
                        nc.sync.dma_start(
                            wq_sb[:, d, :], wqkv_d[d * P : (d + 1) * P, :]
                        )
                    xt_t = xtp.tile([P, CH], f32r, tag="xt", name="xt_t")
                    nc.sync.dma_start(
                        xt_t[:], xt_d[d * P : (d + 1) * P, j * CH : (j + 1) * CH]
                    )
                    xts.append(xt_t)
                for c in range(3):
                    ps = qyp.tile([P, CH], f32, tag="qy", name=f"qkvps{c}")
                    for d in range(ND):
                        nc.tensor.matmul(
                            ps[:],
                            wq_sb[:, d, c * P : (c + 1) * P],
                            xts[d][:],
                            start=(d == 0),
                            stop=(d == ND - 1),
                        )
                    dest = [qt_sb, kt_sb, vt_sb][c]
                    nc.vector.tensor_scalar(
                        out=dest[:, j * CH : (j + 1) * CH],
                        in0=ps[:],
                        scalar1=bq_sb[:, c : c + 1],
                        scalar2=None,
                        op0=ADD,
                    )

            emit_qkv(0)
            if NCHUNK > 1:
                emit_qkv(1)

            # cold-path constants: needed only ~10us in, after the first exp
            nc.sync.dma_start(masks_sb[:], masks_d[:])
            nc.sync.dma_start(ones_sb[64:65, :], ones_d[:])
            nc.sync.dma_start(wo_sb[:], wo_d[:])

            def emit_vhat(j):
                """V^T -> V-hat: f32 stream-transpose, then copy-round to f32r."""
                for h in range(HPC):
                    vtmp = vtp.tile([P, 4, 64], f32, tag="vtmp", name="vtmp")
                    for bj in range(2):
                        for bi in range(4):
                            nc.vector.transpose(
                                vtmp[
                                    32 * bi : 32 * bi + 32,
                                    :,
                                    32 * bj : 32 * bj + 32,
                                ],
                                vt_r[
                                    64 * h + 32 * bj : 64 * h + 32 * bj + 32,
                                    4 * j : 4 * j + 4,
                                    bi,
                                    :,
                                ],
                            )
                    nc.vector.tensor_copy(
                        vhat[h][:, 4 * j : 4 * j + 4, 0:64], vtmp[:]
                    )

            for j in range(NCHUNK):
                emit_vhat(j)
                # ---- attention for chunk j (groups pipelined by one) ----
                ntt = 4 * (j + 1)
                av = [
                    avp.tile([P, CH], f32, tag="av", name=f"av{h}")
                    for h in range(HPC)
                ]
                slots = [(tt, h) for tt in range(ntt) for h in range(HPC)]
                groups = [
                    slots[g0 : g0 + GROUP] for g0 in range(0, len(slots), GROUP)
                ]

                def soff(tt):
                    # diagonal tile k=tt-4j: columns s < 128k are fully masked
                    # (capped at 256 so fp32r matmuls keep free-dim >= 256)
                    o = (tt - 4 * j) * P if tt >= 4 * j else 0
                    return min(max(0, o), 2 * P)

                def flush(grp, sc):
                    pt = ptp.tile([P, GROUP * CH], f32r, tag="pt", name="pt")
                    L = len(grp)
                    o0 = soff(grp[0][0])
                    if all(soff(tt) == o0 for tt, _ in grp):
                        # one (possibly strided) exp over the valid columns
                        sc_v = sc[:].rearrange("p (g c) -> p g c", c=CH)
                        pt_v = pt[:].rearrange("p (g c) -> p g c", c=CH)
                        nc.scalar.activation(
                            pt_v[:, 0:L, o0:],
                            sc_v[:, 0:L, o0:],
                            EXP,
                            scale=0.125,
                        )
                    else:
                        for k, (tt, h) in enumerate(grp):
                            o = soff(tt)
                            nc.scalar.activation(
                                pt[:, k * CH + o : (k + 1) * CH],
                                sc[:, k * CH + o : (k + 1) * CH],
                                EXP,
                                scale=0.125,
                            )
                    if grp[0][0] >= 4 * j:  # diagonal: one masked mul for both heads
                        tt = grp[0][0]
                        o = soff(tt)
                        pt_v = pt[:].rearrange("p (g c) -> p g c", c=CH)
                        nc.vector.tensor_mul(
                            pt_v[:, 0:L, o:],
                            pt_v[:, 0:L, o:],
                            masks_sb[:, tt - 4 * j : tt - 4 * j + 1, o:].broadcast_to(
                                [P, L, CH - o]
                            ),
                        )
                    for k, (tt, h) in enumerate(grp):
                        o = soff(tt)
                        nc.tensor.matmul(
                            av[h][0:65, o:],
                            vhat[h][:, tt, 0:65],
                            pt[:, k * CH + o : (k + 1) * CH],
                            start=(tt == 0),
                            stop=(tt == ntt - 1),
                        )

                pending = None
                for grp in groups:
                    sc = scp.tile([P, GROUP * CH], f32, tag="sc", name="sc")
                    for k, (tt, h) in enumerate(grp):
                        o = soff(tt)
                        nc.tensor.matmul(
                            sc[:, k * CH + o : (k + 1) * CH],
                            kt_sb[64 * h : 64 * h + 64, tt * P : (tt + 1) * P],
                            qt_sb[
                                64 * h : 64 * h + 64, j * CH + o : (j + 1) * CH
                            ],
                            start=True,
                            stop=True,
                        )
                    if pending is not None:
                        flush(*pending)
                    pending = (grp, sc)
                if pending is not None:
                    flush(*pending)

                # ---- reciprocals + numerator copies (free the av tiles) ----
                rcs, nms = [], []
                for h in range(HPC):
                    rc = rcp.tile([P, CH], f32r, tag="rc", name="rc")
                    with nc.allow_low_precision("fp32r recip feeds fp22 matmul"):
                        nc.vector.reciprocal(rc[64:65, :], av[h][64:65, :])
                    nm = otp.tile([HD, CH], f32, tag="nm", name="nm")
                    nc.vector.tensor_copy(nm[:], av[h][0:64, :])
                    rcs.append(rc)
                    nms.append(nm)

                # ---- chunk j+2's QKV keeps PE busy during the div chain ----
                if j + 2 < NCHUNK:
                    emit_qkv(j + 2)

                # ---- denominator broadcast + divide ----
                ot = otp.tile([P, CH], f32r, tag="ot", name="ot")
                for h in range(HPC):
                    bc = qyp.tile([HD, CH], f32, tag="qy", name="bc")
                    nc.tensor.matmul(
                        bc[:],
                        ones_sb[64:65, 0:HD],
                        rcs[h][64:65, :],
                        start=True,
                        stop=True,
                    )
                    nc.vector.tensor_mul(
                        ot[64 * h : 64 * h + 64, :], nms[h][:], bc[:]
                    )

                # ---- output projection for chunk j ----
                for e in range(ND):
                    yt_ps = qyp.tile([P, CH], f32, tag="qy", name="ytps")
                    nc.tensor.matmul(
                        yt_ps[:],
                        wo_sb[:, e * P : (e + 1) * P],
                        ot[:],
                        start=True,
                        stop=True,
                    )
                    yt_sb = ytp.tile([P, CH], f32, tag="yt", name="ytsb")
                    nc.vector.tensor_copy(yt_sb[:], yt_ps[:])
                    nc.sync.dma_start(
                        yt_d[e * P : (e + 1) * P, j * CH : (j + 1) * CH],
                        yt_sb[:],
                    )

    return nc


@functools.lru_cache(maxsize=2)
def _get_nc(S):
    nc = build_nc(S)
    nc.compile()
    return nc


def make_in_maps(input, Wqkv, bqkv, Wo, S):
    """Host-side shard prep. input [1,S,D] (or [S,D]); returns per-core dicts."""
    x = np.asarray(input, dtype=np.float32).reshape(S, D)
    xt = np.ascontiguousarray(x.T)
    Wqkv = np.asarray(Wqkv, dtype=np.float32)
    bqkv = np.asarray(bqkv, dtype=np.float32)
    Wo = np.asarray(Wo, dtype=np.float32)

    # causal masks for the 4 diagonal 128-blocks of a 512 chunk
    pp = np.arange(P)[:, None]
    ff = np.arange(CH)[None, :]
    masks = np.stack(
        [(ff >= pp + P * k).astype(BF16) for k in range(4)], axis=1
    )  # [128, 4, 512]
    masks = np.ascontiguousarray(masks)

    Wq, Wk, Wv = Wqkv[:, 0:D], Wqkv[:, D : 2 * D], Wqkv[:, 2 * D : 3 * D]
    bq, bk, bv = bqkv[0:D], bqkv[D : 2 * D], bqkv[2 * D : 3 * D]

    in_maps = []
    for c in range(NCORES):
        hs = [c * HPC + i for i in range(HPC)]
        cols = lambda W: np.concatenate(
            [W[:, h * HD : (h + 1) * HD] for h in hs], axis=1
        )
        colsb = lambda b: np.concatenate(
            [b[h * HD : (h + 1) * HD] for h in hs], axis=0
        )
        wqkv_l = np.ascontiguousarray(
            np.concatenate([cols(Wq), cols(Wk), cols(Wv)], axis=1)
        )
        bqkv_l = np.ascontiguousarray(
            np.concatenate([colsb(bq), colsb(bk), colsb(bv)], axis=0)
        )
        wo_l = np.ascontiguousarray(Wo[hs[0] * HD : hs[0] * HD + HPC * HD, :])
        in_maps.append(
            {
                "xt": xt,
                "wqkv": wqkv_l,
                "bqkv": bqkv_l,
                "wo": wo_l,
                "masks": masks,
                "ones": np.ones((1, HD), dtype=np.float32),
            }
        )
    return in_maps


def kernel(input, Wqkv, bqkv, Wo, bo):
    from concourse.bass_utils import run_bass_kernel_spmd

    S = np.asarray(input).reshape(-1, D).shape[0]
    nc = _get_nc(S)
    in_maps = make_in_maps(input, Wqkv, bqkv, Wo, S)
    res = None
    last_exc = None
    for _attempt in range(3):  # transient NRT/device errors: retry
        try:
            res = run_bass_kernel_spmd(nc, in_maps, core_ids=list(range(NCORES)))
            break
        except Exception as e:  # noqa: BLE001
            last_exc = e
    if res is None:
        raise last_exc
    yt = res.results[0]["yt"].copy()
    for r in res.results[1:]:
        yt += r["yt"]
    y = yt.T + np.asarray(bo, dtype=np.float32)[None, :]
    return np.ascontiguousarray(y, dtype=np.float32).reshape(1, S, D)

